# revision 12
# baseline (speedup 1.0000x reference)
"""Trainium2 Bass kernel for a single transformer encoder layer with
Music-Transformer relative position attention (causal).

Sharding over 8 NeuronCores:
  - Attention: data-parallel over batch (2) x tensor-parallel over head
    pairs (4) -> core c handles batch c//4, heads {2g, 2g+1}, g = c%4.
  - ctx column-slices are AllGather'd within each 4-core group.
  - LayerNorm + FFN: row-parallel, core c handles rows [512g, 512g+512)
    of its batch; output assembled on host.

Key trick: the Music-Transformer skew is a single SBUF->SBUF DMA per
(head, row-block) with a flat access pattern whose outer step is
(row_stride - 1) elements, which reads W[i, off - i + j] directly.
"""

import numpy as np

import concourse.bass as bass
import concourse.mybir as mybir
import concourse.tile as tile
from concourse import bacc
from concourse.bass import ts
from concourse.bass_utils import run_bass_kernel_spmd
from concourse.masks import make_identity

B, S, D, H, DH, FFN = 2, 2048, 512, 8, 64, 2048
EPS = 1e-5
NCORES = 8
GROUPS = [[0, 1, 2, 3], [4, 5, 6, 7]]
P = 128          # partitions
KB = D // P      # 4 contraction blocks for d_model
NI = S // P      # 16 row blocks
RT = 4           # row tiles per core in FFN phase (512 rows)
NF = FFN // P    # 16 ffn blocks

f32 = mybir.dt.float32
f32r = mybir.dt.float32r
f16 = mybir.dt.float16

_COMPILED = {}


def build_nc():
    nc = bacc.Bacc(None, num_devices=NCORES)

    # ---- per-core DRAM inputs (host pre-sliced / pre-transposed) ----
    xT = nc.dram_tensor("xT", [D, S], f32r, kind="ExternalInput")       # x[b].T
    wq = nc.dram_tensor("wq", [D, P], f32r, kind="ExternalInput")       # /8 folded
    wk = nc.dram_tensor("wk", [D, P], f32r, kind="ExternalInput")
    wv = nc.dram_tensor("wv", [D, P], f32r, kind="ExternalInput")
    bqkv = nc.dram_tensor("bqkv", [3, P], f32, kind="ExternalInput")    # bq/8, bk, bv
    ert = nc.dram_tensor("ert", [DH, S], f32r, kind="ExternalInput")    # Er.T
    xres = nc.dram_tensor("xres", [512, D], f32, kind="ExternalInput")  # row slice
    w1 = nc.dram_tensor("w1", [D, FFN], f32r, kind="ExternalInput")
    w2 = nc.dram_tensor("w2", [FFN, D], f32r, kind="ExternalInput")
    b1 = nc.dram_tensor("b1", [P, NF], f32, kind="ExternalInput")       # transposed
    lnp = nc.dram_tensor("lnp", [5, D], f32, kind="ExternalInput")      # g1,be1,g2,be2,b2
    y = nc.dram_tensor("y", [512, D], f32, kind="ExternalOutput")

    with tile.TileContext(nc) as tc:
        with tc.tile_pool(name="persist", bufs=1) as pp, \
             tc.tile_pool(name="dram", bufs=1, space="DRAM") as dp:

            ccin = dp.tile([S, P], f32)
            ccout = dp.tile([4, S, P], f32)

            qT = pp.tile([P, S], f32r)     # 2 heads stacked on partitions
            kT = pp.tile([P, S], f32r)
            v16 = pp.tile([P, NI, P], f16)  # v natural: [keys, kblock, dh*2]
            ident16 = pp.tile([P, P], f16)
            make_identity(nc, ident16)
            # ErT replicated in both partition halves so it can pair with
            # either head's qT slice (matmul requires equal base partitions)
            ert_sb = pp.tile([P, S], f32r)
            nc.sync.dma_start(out=ert_sb[0:DH, :], in_=ert[:])
            nc.sync.dma_start(out=ert_sb[DH:P, :], in_=ert[:])

            # ---------------- Phase 0: projections ----------------
            with tc.tile_pool(name="p0", bufs=1) as p0, \
                 tc.tile_pool(name="p0ps", bufs=2, space="PSUM") as p0ps:
                xT_sb = p0.tile([P, KB, S], f32r)
                nc.sync.dma_start(out=xT_sb,
                                  in_=xT.rearrange("(kk p) s -> p kk s", p=P))
                w_sb = {}
                for nm, t in (("q", wq), ("k", wk), ("v", wv)):
                    w_sb[nm] = p0.tile([P, KB, P], f32r, tag=f"w{nm}",
                                       name=f"w{nm}_sb")
                    nc.sync.dma_start(out=w_sb[nm],
                                      in_=t.rearrange("(kk p) m -> p kk m", p=P))
                vT16 = p0.tile([P, S], f16)
                # bias APs: per-partition scalars need partition-major layout
                btile = p0.tile([P, 3], f32)
                for i in range(3):
                    nc.sync.dma_start(out=btile[:, i:i + 1], in_=bqkv[i, :])
                for idx, (nm, dst) in enumerate((("q", qT), ("k", kT), ("v", None))):
                    for n in range(S // 512):
                        ps = p0ps.tile([P, 512], f32, tag="pp")
                        for kk in range(KB):
                            nc.tensor.matmul(ps, w_sb[nm][:, kk, :],
                                             xT_sb[:, kk, ts(n, 512)],
                                             start=(kk == 0), stop=(kk == KB - 1))
                        if nm == "v":
                            nc.vector.tensor_scalar_add(
                                out=vT16[:, ts(n, 512)], in0=ps,
                                scalar1=btile[:, idx:idx + 1])
                        else:
                            nc.vector.tensor_scalar_add(
                                out=dst[:, ts(n, 512)], in0=ps,
                                scalar1=btile[:, idx:idx + 1])
                # v natural via PE transpose of vT16
                for t in range(NI):
                    trp = p0ps.tile([P, P], f16, tag="ptr")
                    nc.tensor.transpose(trp, vT16[:, ts(t, P)], ident16)
                    nc.scalar.copy(out=v16[:, t, :], in_=trp)

            # ---------------- Phase 1: attention ----------------
            with tc.tile_pool(name="p1", bufs=2) as p1, \
                 tc.tile_pool(name="p1s", bufs=3) as p1s, \
                 tc.tile_pool(name="p1ps", bufs=2, space="PSUM") as p1ps:
                for hp in range(2):
                    h0 = DH * hp
                    for I in range(NI):
                        LI = P * (I + 1)
                        e0 = S - LI
                        nch = (LI + 511) // 512
                        w16 = p1.tile([P, S], f16, tag="w16")
                        for m0 in range(0, LI, 512):
                            ml = min(512, LI - m0)
                            pw = p1ps.tile([P, 512], f32, tag="pw")
                            nc.tensor.matmul(pw[:, :ml],
                                             qT[h0:h0 + DH, ts(I, P)],
                                             ert_sb[h0:h0 + DH,
                                                    e0 + m0:e0 + m0 + ml],
                                             start=True, stop=True)
                            nc.scalar.copy(out=w16[:, m0:m0 + ml], in_=pw[:, :ml])
                        # skew read: srel[i, j] = w16[i, 127 - i + j]
                        srel = p1.tile([P, S], f16, tag="srel")
                        skew_ap = bass.AP(tensor=w16.tensor,
                                          offset=w16.offset + (P - 1),
                                          ap=[[S - 1, P], [1, LI]])
                        nc.sync.dma_start(out=srel[:, :LI], in_=skew_ap)
                        # causal mask on the diagonal 128 block: fill -1e4
                        nc.gpsimd.affine_select(
                            out=srel[:, LI - P:LI], in_=srel[:, LI - P:LI],
                            base=0, channel_multiplier=1, pattern=[[-1, P]],
                            compare_op=mybir.AluOpType.is_ge, fill=-1e4)
                        sums = p1.tile([P, 4], f32, tag="sums")
                        pctx = p1ps.tile([P, DH], f32, tag="pctx")
                        nblk = I + 1
                        blk = 0
                        for m0 in range(0, LI, 512):
                            ml = min(512, LI - m0)
                            ci = m0 // 512
                            qk = p1ps.tile([P, 512], f32, tag="qk")
                            nc.tensor.matmul(qk[:, :ml],
                                             qT[h0:h0 + DH, ts(I, P)],
                                             kT[h0:h0 + DH, m0:m0 + ml],
                                             start=True, stop=True)
                            sc = p1s.tile([P, 512], f32, tag="sc")
                            nc.vector.tensor_tensor(
                                out=sc[:, :ml], in0=qk[:, :ml],
                                in1=srel[:, m0:m0 + ml],
                                op=mybir.AluOpType.add)
                            aP = p1s.tile([P, 512], f16, tag="aP")
                            nc.scalar.activation(
                                out=aP[:, :ml], in_=sc[:, :ml],
                                func=mybir.ActivationFunctionType.Exp,
                                accum_out=sums[:, ci:ci + 1])
                            for s0 in range(0, ml, P):
                                ptr = p1ps.tile([P, P], f16, tag="ptr")
                                nc.tensor.transpose(ptr, aP[:, s0:s0 + P], ident16)
                                aT = p1s.tile([P, P], f16, tag="aT")
                                if blk % 2 == 0:
                                    nc.scalar.copy(out=aT, in_=ptr)
                                else:
                                    nc.vector.tensor_copy(out=aT, in_=ptr)
                                t = (m0 + s0) // P
                                nc.tensor.matmul(pctx, aT,
                                                 v16[:, t, h0:h0 + DH],
                                                 start=(blk == 0),
                                                 stop=(blk == nblk - 1))
                                blk += 1
                        denom = p1.tile([P, 1], f32, tag="denom")
                        nc.vector.tensor_reduce(out=denom, in_=sums[:, :nch],
                                                axis=mybir.AxisListType.X,
                                                op=mybir.AluOpType.add)
                        nc.vector.reciprocal(out=denom, in_=denom)
                        ctxs = p1.tile([P, DH], f32, tag="ctxs")
                        nc.vector.tensor_scalar_mul(out=ctxs, in0=pctx,
                                                    scalar1=denom)
                        nc.sync.dma_start(
                            out=ccin[ts(I, P), h0:h0 + DH], in_=ctxs)

            # ---------------- Phase 2: AllGather ctx ----------------
            nc.gpsimd.collective_compute(
                "AllGather", mybir.AluOpType.bypass,
                replica_groups=GROUPS,
                ins=[ccin[:].opt()], outs=[ccout[:].opt()])

            # ---------------- Phase 3: LN1 + FFN + LN2 ----------------
            with tc.tile_pool(name="p3", bufs=1) as p3, \
                 tc.tile_pool(name="p3w", bufs=2) as p3w, \
                 tc.tile_pool(name="p3ps", bufs=2, space="PSUM") as p3ps:
                h_sb = p3.tile([P, RT, D], f32)
                ident32 = p3.tile([P, P], f32)
                make_identity(nc, ident32)
                lnp_sb = p3.tile([P, 5, D], f32)
                nc.sync.dma_start(
                    out=lnp_sb,
                    in_=bass.AP(tensor=lnp[:].tensor, offset=0,
                                ap=[[0, P], [D, 5], [1, D]]))
                b1_sb = p3.tile([P, NF], f32)
                nc.sync.dma_start(out=b1_sb, in_=b1[:])
                xr_sb = p3.tile([P, RT, D], f32)
                nc.sync.dma_start(out=xr_sb,
                                  in_=xres.rearrange("(t p) d -> p t d", p=P))
                eps_sb = p3.tile([P, 1], f32)
                nc.vector.memset(eps_sb, EPS)

                # each core reads its own 512-row slice (group rank g = pid%4)
                # from every head-pair column slice of the gathered ctx
                pid = nc.sync.partition_id()
                rsnap = nc.sync.snap((pid % 4) * 512)
                for hp4 in range(4):
                    for t in range(RT):
                        nc.sync.dma_start(
                            out=h_sb[:, t, ts(hp4, P)],
                            in_=ccout[hp4, bass.ds(rsnap + t * P, P), :])

                nc.vector.tensor_tensor(out=h_sb, in0=h_sb, in1=xr_sb,
                                        op=mybir.AluOpType.add)

                def layer_norm(dst, src, t, gamma_i, beta_i, tagp):
                    stats = p3w.tile([P, 6], f32, tag=f"st{tagp}")
                    mv = p3w.tile([P, 2], f32, tag=f"mv{tagp}")
                    nc.vector.bn_stats(out=stats, in_=src)
                    nc.vector.bn_aggr(out=mv, in_=stats)
                    rstd = p3w.tile([P, 1], f32, tag=f"rs{tagp}")
                    nc.scalar.activation(out=rstd, in_=mv[:, 1:2],
                                         func=mybir.ActivationFunctionType.Sqrt,
                                         bias=eps_sb, scale=1.0)
                    nc.vector.reciprocal(out=rstd, in_=rstd)
                    nc.vector.tensor_scalar(out=dst, in0=src,
                                            scalar1=mv[:, 0:1], scalar2=rstd,
                                            op0=mybir.AluOpType.subtract,
                                            op1=mybir.AluOpType.mult)
                    nc.vector.tensor_tensor(out=dst, in0=dst,
                                            in1=lnp_sb[:, gamma_i, :],
                                            op=mybir.AluOpType.mult)
                    nc.vector.tensor_tensor(out=dst, in0=dst,
                                            in1=lnp_sb[:, beta_i, :],
                                            op=mybir.AluOpType.add)

                h1 = p3.tile([P, RT, D], f32)
                for t in range(RT):
                    layer_norm(h1[:, t, :], h_sb[:, t, :], t, 0, 1, "a")

                # h1T (f32r) via PE transpose
                h1T = p3.tile([P, KB, 512], f32r)
                for t in range(RT):
                    for kk in range(KB):
                        ptr = p3ps.tile([P, P], f32, tag="ptr3")
                        nc.tensor.transpose(ptr, h1[:, t, ts(kk, P)], ident32)
                        nc.scalar.copy(out=h1T[:, kk, ts(t, P)], in_=ptr)

                w1_sb = p3.tile([P, KB, FFN], f32r)
                nc.sync.dma_start(out=w1_sb,
                                  in_=w1.rearrange("(kk p) n -> p kk n", p=P))
                gT = p3.tile([P, NF, 512], f32r)
                for f in range(NF):
                    pg = p3ps.tile([P, 512], f32, tag="pg")
                    for kk in range(KB):
                        nc.tensor.matmul(pg, w1_sb[:, kk, ts(f, P)],
                                         h1T[:, kk, :],
                                         start=(kk == 0), stop=(kk == KB - 1))
                    nc.scalar.activation(out=gT[:, f, :], in_=pg,
                                         func=mybir.ActivationFunctionType.Relu,
                                         bias=b1_sb[:, f:f + 1])

                w2_sb = p3.tile([P, NF, D], f32r)
                nc.sync.dma_start(out=w2_sb,
                                  in_=w2.rearrange("(ff p) n -> p ff n", p=P))
                for t in range(RT):
                    po = p3ps.tile([P, D], f32, tag="po")
                    for f in range(NF):
                        nc.tensor.matmul(po, gT[:, f, ts(t, P)], w2_sb[:, f, :],
                                         start=(f == 0), stop=(f == NF - 1))
                    o2 = p3w.tile([P, D], f32, tag="o2")
                    nc.vector.tensor_tensor(out=o2, in0=po, in1=lnp_sb[:, 4, :],
                                            op=mybir.AluOpType.add)
                    nc.vector.tensor_tensor(out=o2, in0=o2, in1=h1[:, t, :],
                                            op=mybir.AluOpType.add)
                    yt = p3w.tile([P, D], f32, tag="yt")
                    layer_norm(yt, o2, t, 2, 3, "b")
                    nc.sync.dma_start(out=y[ts(t, P), :], in_=yt)

    nc.finalize()
    return nc


def _prep_inputs(x, Wq, bq, Wk, bk, Wv, bv, Er, W1, b1, W2, b2, g1, be1, g2, be2):
    x = np.asarray(x, np.float32)
    in_maps = []
    for c in range(NCORES):
        b = c // 4
        g = c % 4
        cols = slice(P * g, P * (g + 1))
        rows = slice(512 * g, 512 * (g + 1))
        m = {
            "xT": np.ascontiguousarray(x[b].T),
            "wq": np.ascontiguousarray(np.asarray(Wq, np.float32)[:, cols] / 8.0),
            "wk": np.ascontiguousarray(np.asarray(Wk, np.float32)[:, cols]),
            "wv": np.ascontiguousarray(np.asarray(Wv, np.float32)[:, cols]),
            "bqkv": np.stack([np.asarray(bq, np.float32)[cols] / 8.0,
                              np.asarray(bk, np.float32)[cols],
                              np.asarray(bv, np.float32)[cols]]),
            "ert": np.ascontiguousarray(np.asarray(Er, np.float32).T),
            "xres": np.ascontiguousarray(x[b, rows]),
            "w1": np.ascontiguousarray(np.asarray(W1, np.float32)),
            "w2": np.ascontiguousarray(np.asarray(W2, np.float32)),
            "b1": np.ascontiguousarray(np.asarray(b1, np.float32).reshape(NF, P).T),
            "lnp": np.stack([np.asarray(g1, np.float32),
                             np.asarray(be1, np.float32),
                             np.asarray(g2, np.float32),
                             np.asarray(be2, np.float32),
                             np.asarray(b2, np.float32)]),
        }
        in_maps.append(m)
    return in_maps


def _get_runner():
    """Build the SPMD jax executable once and cache it."""
    if "runner" in _COMPILED:
        return _COMPILED["runner"]
    import jax
    from jax.experimental.shard_map import shard_map
    from jax.sharding import Mesh, PartitionSpec
    import concourse.mybir as _mybir
    from concourse import bass2jax as b2j

    nc = build_nc()
    b2j.install_neuronx_cc_hook()
    partition_name = (nc.partition_id_tensor.name
                      if nc.partition_id_tensor else None)
    in_names, out_names, out_avals, zero_shapes = [], [], [], []
    for alloc in nc.m.functions[0].allocations:
        if not isinstance(alloc, _mybir.MemoryLocationSet):
            continue
        name = alloc.memorylocations[0].name
        if alloc.kind == "ExternalInput":
            if name != partition_name:
                in_names.append(name)
        elif alloc.kind == "ExternalOutput":
            out_names.append(name)
            shape = tuple(alloc.tensor_shape)
            dtype = _mybir.dt.np(alloc.dtype)
            out_avals.append(jax.core.ShapedArray(shape, dtype))
            zero_shapes.append((shape, dtype))
    n_params = len(in_names)
    n_outs = len(out_avals)
    all_names = in_names + out_names
    if partition_name is not None:
        all_names = all_names + [partition_name]
    donate = tuple(range(n_params, n_params + n_outs))

    def _body(*args):
        operands = list(args)
        if partition_name is not None:
            operands.append(b2j.partition_id_tensor())
        return tuple(b2j._bass_exec_p.bind(
            *operands, out_avals=tuple(out_avals), in_names=tuple(all_names),
            out_names=tuple(out_names), lowering_input_output_aliases=(),
            sim_require_finite=True, sim_require_nnan=True, nc=nc))

    devices = jax.devices()[:NCORES]
    mesh = Mesh(np.asarray(devices), ("core",))
    in_specs = (PartitionSpec("core"),) * (n_params + n_outs)
    out_specs = (PartitionSpec("core"),) * len(out_names)
    sharded = jax.jit(shard_map(_body, mesh=mesh, in_specs=in_specs,
                                out_specs=out_specs, check_rep=False),
                      donate_argnums=donate, keep_unused=True)

    def runner(in_maps):
        concat_in = [np.concatenate([np.asarray(in_maps[c][n])
                                     for c in range(NCORES)], axis=0)
                     for n in in_names]
        concat_zeros = [np.zeros((NCORES * s[0], *s[1:]), d)
                        for s, d in zero_shapes]
        out_arrs = sharded(*concat_in, *concat_zeros)
        return [{name: np.asarray(out_arrs[i]).reshape(
                    NCORES, *out_avals[i].shape)[c]
                 for i, name in enumerate(out_names)}
                for c in range(NCORES)]

    _COMPILED["runner"] = runner
    return runner


def kernel(**inputs):
    in_maps = _prep_inputs(**inputs)
    results = _get_runner()(in_maps)
    out = np.empty((B, S, D), np.float32)
    for c in range(NCORES):
        b, g = c // 4, c % 4
        out[b, 512 * g:512 * (g + 1), :] = results[c]["y"]
    return out
